# revision 61
# baseline (speedup 1.0000x reference)
"""Trainium2 Bass kernel for Co-occurrence Infused Multi-Label Attention, v3.

Shards the n_classes (code) axis across 8 NeuronCores. Key changes vs v2:
  - K/V transforms (tanh(H@k_w), tanh(H@v_w)) are token-sharded 8-ways:
    each core computes its 256 tokens, then a DRAM-bounce AllGather
    (collective_compute) replicates wkt/wvp to all cores. Cuts ~37us of
    replicated PE work per core; wkt/wvp are double-buffered across reps
    and the next rep's local transforms + gather are pre-emitted as fill
    during the current rep so the collective latency hides under compute.
  - exp offload: heads z6/z7's exp runs on DVE via a custom 8-stage op
    EXP16_ANT (exp(s) = p(s/16)^16, deg-2 minimax p; q_w/q_b prescaled by
    1/16 host-side, ACT exp uses scale=16). DVE-routed heads are spread
    through the emission order so ACT and DVE consume different psc ring
    slots concurrently.
  - the next rep's qt DMA + Q-stack (qg/qtt/qwall) are likewise
    pre-emitted during the current rep's last chunk (carry_st0).
  - softmax denominators use reciprocal_approx_fast (~18 bits, plenty).
  - output is [CS, B] (class-major); host transposes
Engine balance per rep (sim): PE 129us, DVE 123us, ACT 121us.
NOTE: GPSIMD cannot access PSUM on real HW — keep Pool offloads off.

Per core (c = class shard of 1152, z = head, b = chunk, t = token):
  wkt [zh, t]     = tanh(k_wT @ HT + k_b)
  wvp [t, z*65+h] = [tanh(HT.T @ v_wT + v_b); ones]   (ones via one memset)
  per chunk (w in 512,512,128):
    qgt [tf, c]   = tanh(trans_wT @ QT + b_tr)
    qtt [zh, c]   = q_wT @ qgt + q_b
    qwall [c, zh] = qgt.T @ W_wT          (per 128-c block)
    per b, z: psc [t128, 2*w] = wkt_z.T @ qtt_z ; et = Exp(psc)
    per b, cb, zh(4z): yp[c128, z*128+..] += et_slice.T @ wvp_z  (16 MMs)
               prod = yp_ctx * qwall      (Pool)
               num  = reduce_h prod       (DVE)
               recd = 1 / yp_den          (DVE)
               scr[z] = num*recd          (DVE)
    per b, cb: outT_cb[:, b] = reduce_z scr  (DVE)
"""

import numpy as np
import ml_dtypes

# deg-2 minimax fit of e^u on [-0.375, 0.375]; exp(s) = p(s/16)^16 with
# p = (EXP_C2*u + EXP_C1)*u + EXP_C0. 8 DVE ALU stages (2 mul-add + 4 sq).
EXP_C2 = 0.4956072753534313
EXP_C1 = 1.0173484236728048
EXP_C0 = 1.000604994235351
_EXP16_OP = None


def _register_exp16():
    """Register the custom DVE op EXP16_ANT (poly-exp) at runtime."""
    global _EXP16_OP
    if _EXP16_OP is not None:
        return _EXP16_OP
    import concourse.dve_ops as dops
    from concourse.dve_ops import DveOp
    from concourse.dve_spec import Spec, Src0, C0, C1, C2, lower, sq, _has_src1
    from concourse.dve_uop import DveOpSpec

    name = "EXP16_ANT"
    if name in dops._SUB_OPCODE_FOR_NAME:
        _EXP16_OP = next(op for op in dops.OPS if op.name == name)
        return _EXP16_OP

    body = (Src0 * C0 + C1) * Src0 + C2
    for _ in range(4):
        body = sq(body)

    def _ref(in0, in1, c0, c1, c2):
        u = np.asarray(in0, np.float32)
        p = ((np.float32(c0) * u + np.float32(c1)) * u + np.float32(c2)).astype(
            np.float32)
        for _ in range(4):
            p = (p * p).astype(np.float32)
        return p

    spec = Spec(body=body, reference=_ref)
    dops._SUB_OPCODE_FOR_NAME[name] = (
        max(dops._SUB_OPCODE_FOR_NAME.values()) + 1
    )
    shas = {}
    for ver in ("v3", "v4"):
        s = DveOpSpec(
            name=name,
            opcode=dops.get_dve_sub_opcode(name),
            uops=lower(spec, ver=ver),
            rd1_en=_has_src1(spec),
        )
        shas[ver] = s.sha(ver)
    op = DveOp(name, spec, subdim=False, uops_sha=shas)
    dops.OPS.append(op)
    dops.CUSTOM_DVE_SPECS[name] = spec
    _EXP16_OP = op
    return op


C_FULL = 8929
D = 768          # d_model
TF = 512         # transform dim (= NH * DK)
NH = 8           # heads
DK = 64          # head dim
B = 4            # chunks
T = 512          # tokens per chunk
BT = B * T       # 2048
N_CORES = 8
CP = 9216        # padded classes (8 * 1152)
CS = CP // N_CORES   # 1152 classes per core
NDC = D // 128       # 6 d-model chunks
NFC = TF // 128      # 4 transform chunks
NTT = BT // 128      # 16 token tiles
C_CHUNKS = [(0, 512), (512, 512), (1024, 128)]

_BF = ml_dtypes.bfloat16

_CACHE = {}


def _build(a_zero: bool, reps: int = 1, et_bufs: int = 34, pop_rate: int = 1,
           dve_zp=(2, 3), tail_mul_pool=True, pool_q=True,
           tail_lag=2, scps_bufs=2, chainp_bufs=1, yp_bufs=3,
           chunk_order=(0, 1, 2), pop_rate_small=3, kv_split=False,
           z_order_cfg=None, boundary_flush=True, dve_small=True,
           qg_bufs=2, carry_pos=None):
    # dve_zp: z-pair indices (z//2) whose exp runs on DVE via EXP16_ANT
    # (only in the a_zero fast path); the rest run exact Exp on ACT.
    # tail_mul_pool: run the ctx*qwall mul on GpSimd instead of DVE.
    # tail_lag: ctx tails are emitted this many fill-pops after their MMs.
    assert tail_lag == 0 or (1 <= tail_lag <= max(yp_bufs, 1) - 1), \
        "tail_lag must stay below yp_bufs (yp buffer recycled by later pops)"
    exp16 = _register_exp16() if (a_zero and dve_zp) else None
    from collections import deque
    from contextlib import ExitStack
    import concourse.bass as bass
    import concourse.mybir as mybir
    import concourse.tile as tile
    from concourse import bacc

    bf = mybir.dt.bfloat16
    f32 = mybir.dt.float32
    AF = mybir.ActivationFunctionType
    ALU = mybir.AluOpType

    nc = bacc.Bacc()

    TS = BT // N_CORES       # 256 tokens handled locally under kv_split
    KV_W = NFC * (TS // 1)   # wkt contribution cols (jz-major, 256 t each)
    KV_COLS = NFC * TS + 2 * 520   # 1024 wkt + 1040 wvp contribution cols

    qt_d = nc.declare_dram_parameter("qt", [D, CS], bf, isOutput=False)
    if kv_split:
        ht_d = nc.declare_dram_parameter("hts", [D, TS], bf, isOutput=False)
    else:
        ht_d = nc.declare_dram_parameter("ht", [D, BT], bf, isOutput=False)
    wtr_d = nc.declare_dram_parameter("wtr", [D, TF], bf, isOutput=False)
    wq_d = nc.declare_dram_parameter("wq", [TF, TF], bf, isOutput=False)
    wk_d = nc.declare_dram_parameter("wk", [D, TF], bf, isOutput=False)
    wv_d = nc.declare_dram_parameter("wv", [D, TF], bf, isOutput=False)
    ww_d = nc.declare_dram_parameter("ww", [TF, TF], bf, isOutput=False)
    btr_d = nc.declare_dram_parameter("btr", [TF], f32, isOutput=False)
    bq_d = nc.declare_dram_parameter("bq", [TF], f32, isOutput=False)
    bk_d = nc.declare_dram_parameter("bk", [TF], f32, isOutput=False)
    bvb_d = nc.declare_dram_parameter("bvb", [1, TF], bf, isOutput=False)
    ab_d = None
    if not a_zero:
        ab_d = nc.declare_dram_parameter("ab", [128, NTT], f32, isOutput=False)
    out_d = nc.declare_dram_parameter("out", [CS, B], f32, isOutput=True)

    with tile.TileContext(nc) as tc, ExitStack() as top:
        const = top.enter_context(tc.tile_pool(name="const", bufs=1))

        w_tr = const.tile([128, NDC * TF], bf)
        w_k = const.tile([128, NDC * TF], bf)
        w_v = const.tile([128, NDC * TF], bf)
        w_q = const.tile([128, NFC * TF], bf)
        w_W = const.tile([128, NFC * TF], bf)
        ht_sb = const.tile([128, NDC * (TS if kv_split else BT)], bf)
        b_tr = const.tile([128, NFC], f32)
        b_q = const.tile([128, NFC], f32)
        b_k = const.tile([128, NFC], f32)
        bvb = const.tile([1, TF], bf)
        ones1 = const.tile([1, 128], bf)
        kv_out = None
        dram = None
        kvp = None
        if kv_split:
            # per-core K/V contribution: [zh, 4jz x 256t | 2jt x (8z x 65)]
            kv_out = const.tile([128, KV_COLS], bf)
            nc.gpsimd.memset(kv_out[:, NFC * TS:], 1.0)  # ones (den) cols
            dram = top.enter_context(tc.tile_pool(name="dram", bufs=2,
                                                  space="DRAM"))
            # wkt/wvp double-buffered across reps: rep r pulls into one
            # buffer while rep r-1's scores still read the other.
            kvp = top.enter_context(tc.tile_pool(name="kvp", bufs=2))
            wkt = wvp = None
        else:
            wkt = const.tile([128, NFC * BT], bf)
            wvp = const.tile([128, NTT * 520], bf)

        # first-chunk qt lives in the const pool so its DMA can lead the sync
        # queue (Qg is the first PE work)
        qt0_sb = const.tile([128, NDC * 512], bf)
        c0_first, w0 = C_CHUNKS[chunk_order[0]]
        if kv_split:
            # tiny local ht slice first, then qt0/w_tr for the qg chain
            nc.sync.dma_start(ht_sb[:].rearrange("p (c x) -> p c x", x=TS),
                              ht_d[:].rearrange("(c p) x -> p c x", p=128))
        nc.sync.dma_start(qt0_sb[:].rearrange("p (c x) -> p c x", x=512)[:, :, 0:w0],
                          qt_d[:, c0_first:c0_first + w0].rearrange(
                              "(c p) x -> p c x", p=128))
        nc.sync.dma_start(w_tr[:].rearrange("p (c x) -> p c x", x=TF),
                          wtr_d[:].rearrange("(c p) x -> p c x", p=128))
        if not kv_split:
            nc.sync.dma_start(ht_sb[:].rearrange("p (c x) -> p c x", x=BT),
                              ht_d[:].rearrange("(c p) x -> p c x", p=128))
        # scalar queue: biases, w_k (so wkt can start early), then the rest
        nc.scalar.dma_start(b_tr[:], btr_d[:].rearrange("(c p) -> p c", p=128))
        nc.scalar.dma_start(b_k[:], bk_d[:].rearrange("(c p) -> p c", p=128))
        nc.scalar.dma_start(w_k[:].rearrange("p (c x) -> p c x", x=TF),
                            wk_d[:].rearrange("(c p) x -> p c x", p=128))
        if kv_split:
            nc.scalar.dma_start(w_v[:].rearrange("p (c x) -> p c x", x=TF),
                                wv_d[:].rearrange("(c p) x -> p c x", p=128))
            nc.scalar.dma_start(bvb[:], bvb_d[:, :])
        nc.scalar.dma_start(w_q[:].rearrange("p (c x) -> p c x", x=TF),
                            wq_d[:].rearrange("(c p) x -> p c x", p=128))
        nc.scalar.dma_start(b_q[:], bq_d[:].rearrange("(c p) -> p c", p=128))
        if not kv_split:
            nc.scalar.dma_start(w_v[:].rearrange("p (c x) -> p c x", x=TF),
                                wv_d[:].rearrange("(c p) x -> p c x", p=128))
            nc.scalar.dma_start(bvb[:], bvb_d[:, :])
        nc.scalar.dma_start(w_W[:].rearrange("p (c x) -> p c x", x=TF),
                            ww_d[:].rearrange("(c p) x -> p c x", p=128))
        nc.gpsimd.memset(ones1[:], 1.0)
        if not kv_split:
            # ones column (h==64 of each z block); tanh writes fill the rest
            nc.gpsimd.memset(wvp[:], 1.0)
        ab_sb = None
        if not a_zero:
            ab_sb = const.tile([128, NTT], f32)
            nc.scalar.dma_start(ab_sb[:], ab_d[:, :])

        with ExitStack() as main:
            qin = main.enter_context(tc.tile_pool(name="qin", bufs=2))
            qg = main.enter_context(tc.tile_pool(name="qg", bufs=qg_bufs))
            # PSUM budget: 8 banks of [128,512]f32:
            #   scps_bufs*2 + chainp_bufs + yp_bufs <= 8
            scps = main.enter_context(tc.tile_pool(
                name="scps", bufs=scps_bufs, space="PSUM"))
            chainp = main.enter_context(tc.tile_pool(
                name="chainp", bufs=chainp_bufs, space="PSUM"))
            yps = (chainp if yp_bufs == 0 else main.enter_context(
                tc.tile_pool(name="yps", bufs=yp_bufs, space="PSUM")))
            etp = main.enter_context(tc.tile_pool(name="etp", bufs=et_bufs))
            prodp = main.enter_context(tc.tile_pool(name="prodp", bufs=3))
            tailp = main.enter_context(tc.tile_pool(name="tailp", bufs=3))
            outp = main.enter_context(tc.tile_pool(name="outp", bufs=2))

            kv_next = None
            carry_st0 = None
            for rep in range(reps):
                if kv_split:
                    if kv_next is None:
                        kv_next = (
                            kvp.tile([128, NFC * BT], bf, tag="wkt", name="wkt"),
                            kvp.tile([128, NTT * 520], bf, tag="wvp", name="wvp"),
                        )
                        kv_cold = True
                    else:
                        kv_cold = False
                    wkt, wvp = kv_next
                    kv_next = None
                fill = deque()
                tailq = deque()

                def pop_tailq(keep):
                    while len(tailq) > keep:
                        tailq.popleft()()

                def pop_fill(k):
                    # trim tails FIRST: a yp buffer is recycled by the fill
                    # unit emitted yp_bufs pops after it, so its tail (the
                    # last reader) must be emitted before that unit.
                    pop_tailq(tail_lag)
                    for _ in range(k):
                        if not fill:
                            return
                        fill.popleft()()

                def flush_fill():
                    while fill:
                        fill.popleft()()
                    pop_tailq(0)

                # ---------- split K/V transforms + AllGather ----------
                def u_lwkt(jh):
                    # local wkt for MY 256 tokens, jz pair (2*jh, 2*jh+1)
                    def f():
                        ps = chainp.tile([128, 512], f32, tag="chain", name="lkps")
                        for ji in range(2):
                            jz = jh * 2 + ji
                            for jd in range(NDC):
                                nc.tensor.matmul(
                                    ps[:, ji * TS: ji * TS + TS],
                                    w_k[:, jd * TF + jz * 128: jd * TF + (jz + 1) * 128],
                                    ht_sb[:, jd * TS: (jd + 1) * TS],
                                    start=(jd == 0), stop=(jd == NDC - 1))
                        for ji in range(2):
                            jz = jh * 2 + ji
                            nc.scalar.activation(
                                kv_out[:, jz * TS: (jz + 1) * TS],
                                ps[:, ji * TS: ji * TS + TS],
                                AF.Tanh, bias=b_k[:, jz:jz + 1])
                    return f

                def u_lwvp(lt):
                    # local wvp for MY token tile lt (global jt = 2*rank+lt)
                    def f():
                        ps = chainp.tile([128, 512], f32, tag="chain", name="lvps")
                        for jd in range(NDC):
                            nc.tensor.matmul(
                                ps[:, 0:512],
                                ht_sb[:, jd * TS + lt * 128: jd * TS + (lt + 1) * 128],
                                w_v[:, jd * TF:(jd + 1) * TF],
                                start=(jd == 0), stop=False)
                        nc.tensor.matmul(ps[:, 0:512], ones1[0:1, :], bvb[0:1, :],
                                         start=False, stop=True)
                        kv_z = kv_out[:, NFC * TS + lt * 520:
                                      NFC * TS + (lt + 1) * 520].rearrange(
                            "p (z h) -> p z h", h=65)
                        nc.scalar.activation(
                            kv_z[:, :, 0:64],
                            ps[:, 0:512].rearrange("p (z h) -> p z h", h=64),
                            AF.Tanh)
                    return f

                def emit_kv_gather(wkt_t, wvp_t):
                    ib = dram.tile([128, KV_COLS], bf, tag="ib", name="ib")
                    ob = dram.tile([128 * N_CORES, KV_COLS], bf, tag="ob",
                                   name="ob")
                    nc.gpsimd.dma_start(ib[:], kv_out[:])
                    nc.gpsimd.collective_compute(
                        "AllGather", mybir.AluOpType.bypass,
                        replica_groups=[list(range(N_CORES))],
                        ins=[ib.opt()], outs=[ob.opt()])
                    obv = ob[:].rearrange("(r p) x -> p r x", p=128)
                    nc.sync.dma_start(
                        wkt_t[:].rearrange("p (r x) -> p r x", x=NFC * TS),
                        obv[:, :, 0:NFC * TS])
                    nc.scalar.dma_start(
                        wvp_t[:].rearrange("p (r x) -> p r x", x=2 * 520),
                        obv[:, :, NFC * TS:])

                def wkt_cols(jz, gt):
                    """wkt col offset for head-block jz, global token tile gt
                    (128 wide) under either layout."""
                    if kv_split:
                        return (gt // 2) * (NFC * TS) + jz * TS + (gt % 2) * 128
                    return jz * BT + gt * 128

                # ---------- transform units (fine-grained) ----------
                def u_wkt(jz, jp):
                    def f():
                        ps = scps.tile([128, 1024], f32, tag="psc", name="wkps")
                        for half in range(2):
                            jt = jp * 2 + half
                            for jd in range(NDC):
                                nc.tensor.matmul(
                                    ps[:, half * 512: half * 512 + 512],
                                    w_k[:, jd * TF + jz * 128: jd * TF + (jz + 1) * 128],
                                    ht_sb[:, jd * BT + jt * 512: jd * BT + (jt + 1) * 512],
                                    start=(jd == 0), stop=(jd == NDC - 1))
                        nc.scalar.activation(
                            wkt[:, jz * BT + jp * 1024: jz * BT + (jp + 1) * 1024],
                            ps[:, 0:1024], AF.Tanh, bias=b_k[:, jz:jz + 1])
                    return f

                def u_wvp(jt):
                    def f():
                        ps = chainp.tile([128, 512], f32, tag="chain", name="wvps")
                        for jd in range(NDC):
                            nc.tensor.matmul(
                                ps[:, 0:512],
                                ht_sb[:, jd * BT + jt * 128: jd * BT + (jt + 1) * 128],
                                w_v[:, jd * TF:(jd + 1) * TF],
                                start=(jd == 0), stop=False)
                        nc.tensor.matmul(ps[:, 0:512], ones1[0:1, :], bvb[0:1, :],
                                         start=False, stop=True)
                        wvp_z = wvp[:, jt * 520: (jt + 1) * 520].rearrange(
                            "p (z h) -> p z h", h=65)
                        nc.scalar.activation(
                            wvp_z[:, :, 0:64],
                            ps[:, 0:512].rearrange("p (z h) -> p z h", h=64),
                            AF.Tanh)
                    return f

                # ---------- per-chunk phase A units ----------
                def u_qt_dma(st, w):
                    def f():
                        nc.sync.dma_start(
                            st["qt"][:].rearrange("p (c x) -> p c x", x=512)[:, :, 0:w],
                            qt_d[:, st["c0"]:st["c0"] + w].rearrange(
                                "(c p) x -> p c x", p=128))
                    return f

                def u_qg(st, jf, w):
                    def f():
                        ps = chainp.tile([128, 512], f32, tag="chain", name="qgps")
                        for jd in range(NDC):
                            nc.tensor.matmul(
                                ps[:, :w],
                                w_tr[:, jd * TF + jf * 128: jd * TF + (jf + 1) * 128],
                                st["qt"][:, jd * 512: jd * 512 + w],
                                start=(jd == 0), stop=(jd == NDC - 1))
                        nc.scalar.activation(st["qgt"][:, jf * 512: jf * 512 + w],
                                             ps[:, :w], AF.Tanh, bias=b_tr[:, jf:jf + 1])
                    return f

                def u_qtt(st, jz, w):
                    def f():
                        ps = chainp.tile([128, 512], f32, tag="chain", name="qtps")
                        for jf in range(NFC):
                            nc.tensor.matmul(
                                ps[:, :w],
                                w_q[:, jf * TF + jz * 128: jf * TF + (jz + 1) * 128],
                                st["qgt"][:, jf * 512: jf * 512 + w],
                                start=(jf == 0), stop=(jf == NFC - 1))
                        qeng = nc.gpsimd if pool_q else nc.vector
                        qeng.tensor_scalar_add(st["qtt"][:, jz * 512: jz * 512 + w],
                                               ps[:, :w], b_q[:, jz:jz + 1])
                    return f

                def u_qwall(st, cb):
                    def f():
                        ps = chainp.tile([128, 512], f32, tag="chain", name="qwps")
                        for jf in range(NFC):
                            nc.tensor.matmul(
                                ps[:, 0:TF],
                                st["qgt"][:, jf * 512 + cb * 128: jf * 512 + (cb + 1) * 128],
                                w_W[:, jf * TF:(jf + 1) * TF],
                                start=(jf == 0), stop=(jf == NFC - 1))
                        (nc.gpsimd if pool_q else nc.vector).tensor_copy(
                            st["qwall"][:, cb * TF:(cb + 1) * TF], ps[:, 0:TF])
                    return f

                def new_chunk_state(ci):
                    c0, w = C_CHUNKS[ci][0], C_CHUNKS[ci][1]
                    st = {"c0": c0, "w": w, "nb": w // 128, "ci": ci}
                    if ci == chunk_order[0]:
                        st["qt"] = qt0_sb
                    else:
                        st["qt"] = qin.tile([128, NDC * 512], bf, tag="qt", name="qt_sb")
                    st["qgt"] = qg.tile([128, NFC * 512], bf, tag="qgt", name="qgt")
                    st["qtt"] = qg.tile([128, NFC * 512], bf, tag="qtt", name="qtt")
                    st["qwall"] = qg.tile([128, 4 * TF], bf, tag="qwall", name="qwall")
                    st["outts"] = [
                        outp.tile([128, B], f32, tag=f"o{cb}", name=f"outt{cb}")
                        for cb in range(st["nb"])
                    ]
                    return st

                def a_units(st):
                    w = st["w"]
                    us = [u_qt_dma(st, w)]
                    us += [u_qg(st, jf, w) for jf in range(NFC)]
                    us += [u_qtt(st, jz, w) for jz in range(NFC)]
                    us += [u_qwall(st, cb) for cb in range(st["nb"])]
                    return us

                # ---------- attention ----------
                def emit_et(et_ap, psc_ap, zp, wide=True):
                    """exp(16*psc) into et: DVE poly for routed z-pairs,
                    exact ACT Exp otherwise. psc holds s/16. The thin
                    (w=128) chunk keeps exp on ACT when dve_small is False:
                    there DVE is tail-dominated while ACT is underloaded."""
                    if exp16 is not None and zp in dve_zp and (wide or dve_small):
                        nc.vector._custom_dve(
                            exp16, out=et_ap, in0=psc_ap,
                            s0=EXP_C2, s1=EXP_C1, imm2=EXP_C0)
                    else:
                        nc.scalar.activation(et_ap, psc_ap, AF.Exp, scale=16.0)

                # Emission order of heads within a b: DVE-routed heads are
                # spread among ACT heads so both engines chew different psc
                # ring slots CONCURRENTLY (the ring frees slots in emission
                # order, so a run of same-engine tiles serializes on that
                # engine).
                if z_order_cfg is not None:
                    z_order = list(z_order_cfg)
                else:
                    zp_order = [zp for zp in range(NH // 2) if zp in dve_zp] + \
                               [zp for zp in range(NH // 2) if zp not in dve_zp]
                    z_order = [zp * 2 + zi for zp in zp_order for zi in range(2)]
                zp_order = list(dict.fromkeys(z // 2 for z in z_order))

                def emit_scores(st, bb):
                    """scores + exp for all z of (chunk, b); returns ets."""
                    w = st["w"]
                    pr = pop_rate if w > 128 else pop_rate_small
                    ets = {}
                    if w > 128:
                        for z in z_order:
                            jz, hz = z // 2, (z % 2) * 64
                            pair_et = []
                            for half in range(2):
                                psc = scps.tile([128, 1024], f32, tag="psc", name="psc")
                                for slot in range(2):
                                    jt = half * 2 + slot
                                    wc = wkt_cols(jz, bb * 4 + jt)
                                    nc.tensor.matmul(
                                        psc[:, slot * w: slot * w + w],
                                        wkt[hz:hz + 64, wc: wc + 128],
                                        st["qtt"][hz:hz + 64, jz * 512: jz * 512 + w],
                                        start=True, stop=True)
                                et = etp.tile([128, 1024], bf, tag="et", name="et")
                                if a_zero:
                                    emit_et(et[:, 0:2 * w], psc[:, 0:2 * w], z // 2)
                                else:
                                    for slot in range(2):
                                        jt = half * 2 + slot
                                        gt = bb * 4 + jt
                                        nc.scalar.activation(
                                            et[:, slot * w: slot * w + w],
                                            psc[:, slot * w: slot * w + w],
                                            AF.Exp, scale=16.0,
                                            bias=ab_sb[:, gt:gt + 1])
                                pair_et.append(et)
                                pop_fill(pr)
                            ets[z] = pair_et
                    else:
                        for zp in zp_order:
                            psc = scps.tile([128, 1024], f32, tag="psc", name="psc")
                            for zi in range(2):
                                z = zp * 2 + zi
                                jz, hz = z // 2, (z % 2) * 64
                                for jt in range(4):
                                    s = zi * 4 + jt
                                    wc = wkt_cols(jz, bb * 4 + jt)
                                    nc.tensor.matmul(
                                        psc[:, s * 128: (s + 1) * 128],
                                        wkt[hz:hz + 64, wc: wc + 128],
                                        st["qtt"][hz:hz + 64, jz * 512: jz * 512 + w],
                                        start=True, stop=True)
                            et = etp.tile([128, 1024], bf, tag="et", name="et")
                            if a_zero:
                                emit_et(et[:], psc[:], zp, wide=False)
                            else:
                                for zi in range(2):
                                    for jt in range(4):
                                        s = zi * 4 + jt
                                        gt = bb * 4 + jt
                                        nc.scalar.activation(
                                            et[:, s * 128: (s + 1) * 128],
                                            psc[:, s * 128: (s + 1) * 128],
                                            AF.Exp, scale=16.0,
                                            bias=ab_sb[:, gt:gt + 1])
                            ets[zp] = et
                            pop_fill(pr)
                    return ets

                def u_ctx(st, bb, cb, zh, ets, scrs):
                    """context MMs for one (b, c-block, z-half: 4 z); the
                    elementwise tail is deferred via tailq (lagged) so its
                    DVE ops never head-of-line-block later exp work."""
                    w = st["w"]

                    def f():
                        yp = yps.tile([128, 512], f32, tag="chain", name="yp")
                        for zi in range(4):
                            z = zh * 4 + zi
                            for jt in range(4):
                                if w > 128:
                                    half, slot = jt // 2, jt % 2
                                    lhsT = ets[z][half][:, slot * w + cb * 128:
                                                        slot * w + (cb + 1) * 128]
                                else:
                                    s = (z % 2) * 4 + jt
                                    lhsT = ets[z // 2][:, s * 128: (s + 1) * 128]
                                gt = bb * 4 + jt
                                nc.tensor.matmul(
                                    yp[:, zi * 128: zi * 128 + 65],
                                    lhsT,
                                    wvp[:, gt * 520 + z * 65: gt * 520 + (z + 1) * 65],
                                    start=(jt == 0), stop=(jt == 3))

                        def tail():
                            ypz = yp[:].rearrange("p (z c) -> p z c", c=128)
                            prod = prodp.tile([128, 256], f32, tag="prod", name="prod")
                            mul_eng = nc.gpsimd if tail_mul_pool else nc.vector
                            mul_eng.tensor_mul(
                                prod[:].rearrange("p (z h) -> p z h", h=64),
                                ypz[:, :, 0:64],
                                st["qwall"][:, cb * TF + zh * 256:
                                            cb * TF + (zh + 1) * 256].rearrange(
                                    "p (z h) -> p z h", h=64))
                            num = tailp.tile([128, 4], f32, tag="num", name="num")
                            nc.vector.tensor_reduce(
                                num[:], prod[:].rearrange("p (z h) -> p z h", h=64),
                                mybir.AxisListType.X, ALU.add)
                            recd = tailp.tile([128, 4], f32, tag="recd", name="recd")
                            # denominators are sums of positive exps (>= ~1):
                            # the fast approx (~18 bits) is plenty.
                            nc.vector.reciprocal_approx_fast(
                                recd[:], ypz[:, :, 64:65])
                            nc.vector.tensor_mul(scrs[cb][:, zh * 4: (zh + 1) * 4],
                                                 num[:], recd[:])
                            if zh == 1:
                                nc.vector.tensor_reduce(
                                    st["outts"][cb][:, bb:bb + 1],
                                    scrs[cb][:].rearrange("p (o z) -> p o z", o=1),
                                    mybir.AxisListType.X, ALU.add)
                                if bb == B - 1:
                                    nc.sync.dma_start(
                                        out_d[st["c0"] + cb * 128:
                                              st["c0"] + (cb + 1) * 128, :],
                                        st["outts"][cb][:, :])
                        tailq.append(tail)
                    return f

                def ctx_units(st, bb, ets):
                    scrs = [tailp.tile([128, 8], f32, tag=f"scr{cb}", name="scr")
                            for cb in range(st["nb"])]
                    return [u_ctx(st, bb, cb, zh, ets, scrs)
                            for cb in range(st["nb"]) for zh in range(2)]

                # ---------- prologue ----------
                if carry_st0 is not None:
                    # phase-A units (qt DMA, qg, qtt, qwall) were pre-emitted
                    # as fill during the previous rep's last chunk.
                    st0 = carry_st0
                    carry_st0 = None
                    prologue_done = True
                else:
                    st0 = new_chunk_state(chunk_order[0])
                    prologue_done = False
                if (not prologue_done) and rep > 0:
                    u_qt_dma(st0, st0["w"])()
                if kv_split:
                    # local K/V transforms + cross-core gather: inline only on
                    # the cold rep; later reps pre-emit this during rep r-1 so
                    # the collective latency hides behind compute.
                    if kv_cold:
                        for jh in range(2):
                            u_lwkt(jh)()
                        for lt in range(2):
                            u_lwvp(lt)()
                        emit_kv_gather(wkt, wvp)
                    if not prologue_done:
                        for jf in range(NFC):
                            u_qg(st0, jf, st0["w"])()
                        u_qtt(st0, 0, st0["w"])()
                        fill.extend([u_qtt(st0, 1, st0["w"]),
                                     u_qtt(st0, 2, st0["w"]),
                                     u_qtt(st0, 3, st0["w"])])
                else:
                    for jf in range(NFC):
                        u_qg(st0, jf, st0["w"])()
                    u_qtt(st0, 0, st0["w"])()
                    u_wkt(0, 0)()
                    u_wkt(0, 1)()
                    fill.extend([u_wkt(1, 0), u_wkt(1, 1), u_qtt(st0, 1, st0["w"]),
                                 u_wkt(2, 0), u_wkt(2, 1), u_qtt(st0, 2, st0["w"]),
                                 u_wkt(3, 0), u_wkt(3, 1), u_qtt(st0, 3, st0["w"])])
                if not prologue_done:
                    fill.extend([u_qwall(st0, cb) for cb in range(st0["nb"])])
                if not kv_split:
                    fill.extend([u_wvp(jt) for jt in range(4)])

                # ---------- main pipeline ----------
                st = st0
                nst = None
                for pos in range(len(chunk_order)):
                    for bb in range(B):
                        ets = emit_scores(st, bb)
                        fill.extend(ctx_units(st, bb, ets))
                        if pos == 0 and bb < 3 and not kv_split:
                            fill.extend(u_wvp(4 * (bb + 1) + k) for k in range(4))
                        if bb == 0 and pos + 1 < len(chunk_order):
                            nst = new_chunk_state(chunk_order[pos + 1])
                            fill.extend(a_units(nst))
                        cpos, cbb = (carry_pos if carry_pos is not None
                                     else (len(chunk_order) - 1, 0))
                        if (pos == cpos and bb == cbb and rep + 1 < reps):
                            carry_st0 = new_chunk_state(chunk_order[0])
                            fill.extend(a_units(carry_st0))
                        if (kv_split and pos == 1 and bb == 0
                                and rep + 1 < reps):
                            kv_next = (
                                kvp.tile([128, NFC * BT], bf, tag="wkt",
                                         name="wkt"),
                                kvp.tile([128, NTT * 520], bf, tag="wvp",
                                         name="wvp"),
                            )
                            nwkt, nwvp = kv_next
                            fill.extend([u_lwkt(0), u_lwkt(1),
                                         u_lwvp(0), u_lwvp(1)])
                            fill.append(
                                lambda w1=nwkt, w2=nwvp: emit_kv_gather(w1, w2))
                    if boundary_flush or pos + 1 >= len(chunk_order):
                        flush_fill()
                    st = nst
                    nst = None

    nc.compile()
    return nc


# the shipping configuration (used by kernel() and test.py).
# NOTE: GPSIMD cannot access PSUM on real HW (BIR verifier), so the Pool
# offloads (tail_mul_pool / pool_q) must stay False.
BUILD_CFG = dict(dve_zp=(3,), z_order_cfg=(0, 1, 6, 2, 3, 7, 4, 5),
                 tail_mul_pool=False, pool_q=False, tail_lag=0,
                 scps_bufs=3, chainp_bufs=2, yp_bufs=0, kv_split=True,
                 et_bufs=30)


def _get_nc(a_zero: bool, reps: int = 1):
    key = ("nc", a_zero, reps)
    if key not in _CACHE:
        _CACHE[key] = _build(a_zero, reps=reps, **BUILD_CFG)
    return _CACHE[key]


def _prep_inputs(Q, H, a, trans_w, trans_b, q_w, q_b, k_w, k_b, v_w, v_b, W_w):
    """Host-side sharding/layout. Returns (in_maps, a_zero)."""
    a = np.asarray(a, np.float32)
    a_zero = not np.any(a)

    qt_full = np.zeros((D, CP), _BF)
    qt_full[:, :C_FULL] = np.asarray(Q, np.float32).T.astype(_BF)
    ht = np.ascontiguousarray(
        np.asarray(H, np.float32).reshape(BT, D).T.astype(_BF))
    shared = {}
    if not BUILD_CFG.get("kv_split"):
        shared["ht"] = ht
    shared.update({
        "wtr": np.ascontiguousarray(np.asarray(trans_w, np.float32).T.astype(_BF)),
        # q_w/q_b prescaled by 1/16: psc holds s/16 (exp is taken with
        # scale=16 on ACT, or the poly exp16 on DVE which wants s/16).
        "wq": np.ascontiguousarray(
            (np.asarray(q_w, np.float32).T / 16.0).astype(_BF)),
        "wk": np.ascontiguousarray(np.asarray(k_w, np.float32).T.astype(_BF)),
        "wv": np.ascontiguousarray(np.asarray(v_w, np.float32).T.astype(_BF)),
        "ww": np.ascontiguousarray(np.asarray(W_w, np.float32).T.astype(_BF)),
        "btr": np.asarray(trans_b, np.float32),
        "bq": np.asarray(q_b, np.float32) / 16.0,
        "bk": np.asarray(k_b, np.float32),
        "bvb": np.asarray(v_b, np.float32).reshape(1, TF).astype(_BF),
    })
    if not a_zero:
        ab = a.reshape(B, 4, 128).transpose(2, 0, 1).reshape(128, NTT)
        shared["ab"] = np.ascontiguousarray(ab.astype(np.float32))
    in_maps = []
    for c in range(N_CORES):
        m = dict(shared)
        m["qt"] = np.ascontiguousarray(qt_full[:, c * CS:(c + 1) * CS])
        if BUILD_CFG.get("kv_split"):
            ts = BT // N_CORES
            m["hts"] = np.ascontiguousarray(ht[:, c * ts:(c + 1) * ts])
        in_maps.append(m)
    return in_maps, a_zero


def kernel(**inputs) -> np.ndarray:
    from concourse.bass_utils import run_bass_kernel_spmd

    in_maps, a_zero = _prep_inputs(**inputs)
    nc = _get_nc(a_zero)
    res = run_bass_kernel_spmd(nc, in_maps, list(range(N_CORES)))
    out = np.concatenate([res.results[c]["out"] for c in range(N_CORES)], axis=0)
    return np.ascontiguousarray(out.T[:, :C_FULL])



# revision 62
# speedup vs baseline: 1.0822x; 1.0822x over previous
"""Trainium2 Bass kernel for Co-occurrence Infused Multi-Label Attention, v3.

Shards the n_classes (code) axis across 8 NeuronCores. Key changes vs v2:
  - K/V transforms (tanh(H@k_w), tanh(H@v_w)) are token-sharded 8-ways:
    each core computes its 256 tokens, then a DRAM-bounce AllGather
    (collective_compute) replicates wkt/wvp to all cores. Cuts ~37us of
    replicated PE work per core; wkt/wvp are double-buffered across reps
    and the next rep's local transforms + gather are pre-emitted as fill
    during the current rep so the collective latency hides under compute.
  - exp offload: heads z6/z7's exp runs on DVE via a custom 8-stage op
    EXP16_ANT (exp(s) = p(s/16)^16, deg-2 minimax p; q_w/q_b prescaled by
    1/16 host-side, ACT exp uses scale=16). DVE-routed heads are spread
    through the emission order so ACT and DVE consume different psc ring
    slots concurrently.
  - the next rep's qt DMA + Q-stack (qg/qtt/qwall) are likewise
    pre-emitted during the current rep's last chunk (carry_st0).
  - softmax denominators use reciprocal_approx_fast (~18 bits, plenty).
  - output is [CS, B] (class-major); host transposes
Engine balance per rep (sim): PE 129us, DVE 123us, ACT 121us.
NOTE: GPSIMD cannot access PSUM on real HW — keep Pool offloads off.

Per core (c = class shard of 1152, z = head, b = chunk, t = token):
  wkt [zh, t]     = tanh(k_wT @ HT + k_b)
  wvp [t, z*65+h] = [tanh(HT.T @ v_wT + v_b); ones]   (ones via one memset)
  per chunk (w in 512,512,128):
    qgt [tf, c]   = tanh(trans_wT @ QT + b_tr)
    qtt [zh, c]   = q_wT @ qgt + q_b
    qwall [c, zh] = qgt.T @ W_wT          (per 128-c block)
    per b, z: psc [t128, 2*w] = wkt_z.T @ qtt_z ; et = Exp(psc)
    per b, cb, zh(4z): yp[c128, z*128+..] += et_slice.T @ wvp_z  (16 MMs)
               prod = yp_ctx * qwall      (Pool)
               num  = reduce_h prod       (DVE)
               recd = 1 / yp_den          (DVE)
               scr[z] = num*recd          (DVE)
    per b, cb: outT_cb[:, b] = reduce_z scr  (DVE)
"""

import numpy as np
import ml_dtypes

# deg-2 minimax fit of e^u on [-0.375, 0.375]; exp(s) = p(s/16)^16 with
# p = (EXP_C2*u + EXP_C1)*u + EXP_C0. 8 DVE ALU stages (2 mul-add + 4 sq).
EXP_C2 = 0.4956072753534313
EXP_C1 = 1.0173484236728048
EXP_C0 = 1.000604994235351
_EXP16_OP = None


def _register_exp16():
    """Register the custom DVE op EXP16_ANT (poly-exp) at runtime."""
    global _EXP16_OP
    if _EXP16_OP is not None:
        return _EXP16_OP
    import concourse.dve_ops as dops
    from concourse.dve_ops import DveOp
    from concourse.dve_spec import Spec, Src0, C0, C1, C2, lower, sq, _has_src1
    from concourse.dve_uop import DveOpSpec

    name = "EXP16_ANT"
    if name in dops._SUB_OPCODE_FOR_NAME:
        _EXP16_OP = next(op for op in dops.OPS if op.name == name)
        return _EXP16_OP

    body = (Src0 * C0 + C1) * Src0 + C2
    for _ in range(4):
        body = sq(body)

    def _ref(in0, in1, c0, c1, c2):
        u = np.asarray(in0, np.float32)
        p = ((np.float32(c0) * u + np.float32(c1)) * u + np.float32(c2)).astype(
            np.float32)
        for _ in range(4):
            p = (p * p).astype(np.float32)
        return p

    spec = Spec(body=body, reference=_ref)
    dops._SUB_OPCODE_FOR_NAME[name] = (
        max(dops._SUB_OPCODE_FOR_NAME.values()) + 1
    )
    shas = {}
    for ver in ("v3", "v4"):
        s = DveOpSpec(
            name=name,
            opcode=dops.get_dve_sub_opcode(name),
            uops=lower(spec, ver=ver),
            rd1_en=_has_src1(spec),
        )
        shas[ver] = s.sha(ver)
    op = DveOp(name, spec, subdim=False, uops_sha=shas)
    dops.OPS.append(op)
    dops.CUSTOM_DVE_SPECS[name] = spec
    _EXP16_OP = op
    return op


C_FULL = 8929
D = 768          # d_model
TF = 512         # transform dim (= NH * DK)
NH = 8           # heads
DK = 64          # head dim
B = 4            # chunks
T = 512          # tokens per chunk
BT = B * T       # 2048
N_CORES = 8
CP = 9216        # padded classes (8 * 1152)
CS = CP // N_CORES   # 1152 classes per core
NDC = D // 128       # 6 d-model chunks
NFC = TF // 128      # 4 transform chunks
NTT = BT // 128      # 16 token tiles
C_CHUNKS = [(0, 512), (512, 512), (1024, 128)]

_BF = ml_dtypes.bfloat16

_CACHE = {}


def _build(a_zero: bool, reps: int = 1, et_bufs: int = 34, pop_rate: int = 1,
           dve_zp=(2, 3), tail_mul_pool=True, pool_q=True,
           tail_lag=2, scps_bufs=2, chainp_bufs=1, yp_bufs=3,
           chunk_order=(0, 1, 2), pop_rate_small=3, kv_split=False,
           z_order_cfg=None, boundary_flush=True, dve_small=True,
           qg_bufs=2, carry_pos=None, qwall_act=False):
    # dve_zp: z-pair indices (z//2) whose exp runs on DVE via EXP16_ANT
    # (only in the a_zero fast path); the rest run exact Exp on ACT.
    # tail_mul_pool: run the ctx*qwall mul on GpSimd instead of DVE.
    # tail_lag: ctx tails are emitted this many fill-pops after their MMs.
    assert tail_lag == 0 or (1 <= tail_lag <= max(yp_bufs, 1) - 1), \
        "tail_lag must stay below yp_bufs (yp buffer recycled by later pops)"
    exp16 = _register_exp16() if (a_zero and dve_zp) else None
    from collections import deque
    from contextlib import ExitStack
    import concourse.bass as bass
    import concourse.mybir as mybir
    import concourse.tile as tile
    from concourse import bacc

    bf = mybir.dt.bfloat16
    f32 = mybir.dt.float32
    AF = mybir.ActivationFunctionType
    ALU = mybir.AluOpType

    nc = bacc.Bacc()

    TS = BT // N_CORES       # 256 tokens handled locally under kv_split
    KV_W = NFC * (TS // 1)   # wkt contribution cols (jz-major, 256 t each)
    KV_COLS = NFC * TS + 2 * 520   # 1024 wkt + 1040 wvp contribution cols

    qt_d = nc.declare_dram_parameter("qt", [D, CS], bf, isOutput=False)
    if kv_split:
        ht_d = nc.declare_dram_parameter("hts", [D, TS], bf, isOutput=False)
    else:
        ht_d = nc.declare_dram_parameter("ht", [D, BT], bf, isOutput=False)
    wtr_d = nc.declare_dram_parameter("wtr", [D, TF], bf, isOutput=False)
    wq_d = nc.declare_dram_parameter("wq", [TF, TF], bf, isOutput=False)
    wk_d = nc.declare_dram_parameter("wk", [D, TF], bf, isOutput=False)
    wv_d = nc.declare_dram_parameter("wv", [D, TF], bf, isOutput=False)
    ww_d = nc.declare_dram_parameter("ww", [TF, TF], bf, isOutput=False)
    btr_d = nc.declare_dram_parameter("btr", [TF], f32, isOutput=False)
    bq_d = nc.declare_dram_parameter("bq", [TF], f32, isOutput=False)
    bk_d = nc.declare_dram_parameter("bk", [TF], f32, isOutput=False)
    bvb_d = nc.declare_dram_parameter("bvb", [1, TF], bf, isOutput=False)
    ab_d = None
    if not a_zero:
        ab_d = nc.declare_dram_parameter("ab", [128, NTT], f32, isOutput=False)
    out_d = nc.declare_dram_parameter("out", [CS, B], f32, isOutput=True)

    with tile.TileContext(nc) as tc, ExitStack() as top:
        const = top.enter_context(tc.tile_pool(name="const", bufs=1))

        w_tr = const.tile([128, NDC * TF], bf)
        w_k = const.tile([128, NDC * TF], bf)
        w_v = const.tile([128, NDC * TF], bf)
        w_q = const.tile([128, NFC * TF], bf)
        w_W = const.tile([128, NFC * TF], bf)
        ht_sb = const.tile([128, NDC * (TS if kv_split else BT)], bf)
        b_tr = const.tile([128, NFC], f32)
        b_q = const.tile([128, NFC], f32)
        b_k = const.tile([128, NFC], f32)
        bvb = const.tile([1, TF], bf)
        ones1 = const.tile([1, 128], bf)
        kv_out = None
        dram = None
        kvp = None
        if kv_split:
            # per-core K/V contribution: [zh, 4jz x 256t | 2jt x (8z x 65)]
            kv_out = const.tile([128, KV_COLS], bf)
            nc.gpsimd.memset(kv_out[:, NFC * TS:], 1.0)  # ones (den) cols
            dram = top.enter_context(tc.tile_pool(name="dram", bufs=2,
                                                  space="DRAM"))
            # wkt/wvp double-buffered across reps: rep r pulls into one
            # buffer while rep r-1's scores still read the other.
            kvp = top.enter_context(tc.tile_pool(name="kvp", bufs=2))
            wkt = wvp = None
        else:
            wkt = const.tile([128, NFC * BT], bf)
            wvp = const.tile([128, NTT * 520], bf)

        # first-chunk qt lives in the const pool so its DMA can lead the sync
        # queue (Qg is the first PE work)
        qt0_sb = const.tile([128, NDC * 512], bf)
        c0_first, w0 = C_CHUNKS[chunk_order[0]]
        if kv_split:
            # tiny local ht slice first, then qt0/w_tr for the qg chain
            nc.sync.dma_start(ht_sb[:].rearrange("p (c x) -> p c x", x=TS),
                              ht_d[:].rearrange("(c p) x -> p c x", p=128))
        nc.sync.dma_start(qt0_sb[:].rearrange("p (c x) -> p c x", x=512)[:, :, 0:w0],
                          qt_d[:, c0_first:c0_first + w0].rearrange(
                              "(c p) x -> p c x", p=128))
        nc.sync.dma_start(w_tr[:].rearrange("p (c x) -> p c x", x=TF),
                          wtr_d[:].rearrange("(c p) x -> p c x", p=128))
        if not kv_split:
            nc.sync.dma_start(ht_sb[:].rearrange("p (c x) -> p c x", x=BT),
                              ht_d[:].rearrange("(c p) x -> p c x", p=128))
        # scalar queue: biases, w_k (so wkt can start early), then the rest
        nc.scalar.dma_start(b_tr[:], btr_d[:].rearrange("(c p) -> p c", p=128))
        nc.scalar.dma_start(b_k[:], bk_d[:].rearrange("(c p) -> p c", p=128))
        nc.scalar.dma_start(w_k[:].rearrange("p (c x) -> p c x", x=TF),
                            wk_d[:].rearrange("(c p) x -> p c x", p=128))
        if kv_split:
            nc.scalar.dma_start(w_v[:].rearrange("p (c x) -> p c x", x=TF),
                                wv_d[:].rearrange("(c p) x -> p c x", p=128))
            nc.scalar.dma_start(bvb[:], bvb_d[:, :])
        nc.scalar.dma_start(w_q[:].rearrange("p (c x) -> p c x", x=TF),
                            wq_d[:].rearrange("(c p) x -> p c x", p=128))
        nc.scalar.dma_start(b_q[:], bq_d[:].rearrange("(c p) -> p c", p=128))
        if not kv_split:
            nc.scalar.dma_start(w_v[:].rearrange("p (c x) -> p c x", x=TF),
                                wv_d[:].rearrange("(c p) x -> p c x", p=128))
            nc.scalar.dma_start(bvb[:], bvb_d[:, :])
        nc.scalar.dma_start(w_W[:].rearrange("p (c x) -> p c x", x=TF),
                            ww_d[:].rearrange("(c p) x -> p c x", p=128))
        nc.gpsimd.memset(ones1[:], 1.0)
        if not kv_split:
            # ones column (h==64 of each z block); tanh writes fill the rest
            nc.gpsimd.memset(wvp[:], 1.0)
        ab_sb = None
        if not a_zero:
            ab_sb = const.tile([128, NTT], f32)
            nc.scalar.dma_start(ab_sb[:], ab_d[:, :])

        with ExitStack() as main:
            qin = main.enter_context(tc.tile_pool(name="qin", bufs=2))
            qg = main.enter_context(tc.tile_pool(name="qg", bufs=qg_bufs))
            # PSUM budget: 8 banks of [128,512]f32:
            #   scps_bufs*2 + chainp_bufs + yp_bufs <= 8
            scps = main.enter_context(tc.tile_pool(
                name="scps", bufs=scps_bufs, space="PSUM"))
            chainp = main.enter_context(tc.tile_pool(
                name="chainp", bufs=chainp_bufs, space="PSUM"))
            yps = (chainp if yp_bufs == 0 else main.enter_context(
                tc.tile_pool(name="yps", bufs=yp_bufs, space="PSUM")))
            etp = main.enter_context(tc.tile_pool(name="etp", bufs=et_bufs))
            prodp = main.enter_context(tc.tile_pool(name="prodp", bufs=3))
            tailp = main.enter_context(tc.tile_pool(name="tailp", bufs=3))
            outp = main.enter_context(tc.tile_pool(name="outp", bufs=2))

            kv_next = None
            carry_st0 = None
            for rep in range(reps):
                if kv_split:
                    if kv_next is None:
                        kv_next = (
                            kvp.tile([128, NFC * BT], bf, tag="wkt", name="wkt"),
                            kvp.tile([128, NTT * 520], bf, tag="wvp", name="wvp"),
                        )
                        kv_cold = True
                    else:
                        kv_cold = False
                    wkt, wvp = kv_next
                    kv_next = None
                fill = deque()
                tailq = deque()

                def pop_tailq(keep):
                    while len(tailq) > keep:
                        tailq.popleft()()

                def pop_fill(k):
                    # trim tails FIRST: a yp buffer is recycled by the fill
                    # unit emitted yp_bufs pops after it, so its tail (the
                    # last reader) must be emitted before that unit.
                    pop_tailq(tail_lag)
                    for _ in range(k):
                        if not fill:
                            return
                        fill.popleft()()

                def flush_fill():
                    while fill:
                        fill.popleft()()
                    pop_tailq(0)

                # ---------- split K/V transforms + AllGather ----------
                def u_lwkt(jh):
                    # local wkt for MY 256 tokens, jz pair (2*jh, 2*jh+1)
                    def f():
                        ps = chainp.tile([128, 512], f32, tag="chain", name="lkps")
                        for ji in range(2):
                            jz = jh * 2 + ji
                            for jd in range(NDC):
                                nc.tensor.matmul(
                                    ps[:, ji * TS: ji * TS + TS],
                                    w_k[:, jd * TF + jz * 128: jd * TF + (jz + 1) * 128],
                                    ht_sb[:, jd * TS: (jd + 1) * TS],
                                    start=(jd == 0), stop=(jd == NDC - 1))
                        for ji in range(2):
                            jz = jh * 2 + ji
                            nc.scalar.activation(
                                kv_out[:, jz * TS: (jz + 1) * TS],
                                ps[:, ji * TS: ji * TS + TS],
                                AF.Tanh, bias=b_k[:, jz:jz + 1])
                    return f

                def u_lwvp(lt):
                    # local wvp for MY token tile lt (global jt = 2*rank+lt)
                    def f():
                        ps = chainp.tile([128, 512], f32, tag="chain", name="lvps")
                        for jd in range(NDC):
                            nc.tensor.matmul(
                                ps[:, 0:512],
                                ht_sb[:, jd * TS + lt * 128: jd * TS + (lt + 1) * 128],
                                w_v[:, jd * TF:(jd + 1) * TF],
                                start=(jd == 0), stop=False)
                        nc.tensor.matmul(ps[:, 0:512], ones1[0:1, :], bvb[0:1, :],
                                         start=False, stop=True)
                        kv_z = kv_out[:, NFC * TS + lt * 520:
                                      NFC * TS + (lt + 1) * 520].rearrange(
                            "p (z h) -> p z h", h=65)
                        nc.scalar.activation(
                            kv_z[:, :, 0:64],
                            ps[:, 0:512].rearrange("p (z h) -> p z h", h=64),
                            AF.Tanh)
                    return f

                def emit_kv_gather(wkt_t, wvp_t):
                    ib = dram.tile([128, KV_COLS], bf, tag="ib", name="ib")
                    ob = dram.tile([128 * N_CORES, KV_COLS], bf, tag="ob",
                                   name="ob")
                    nc.gpsimd.dma_start(ib[:], kv_out[:])
                    nc.gpsimd.collective_compute(
                        "AllGather", mybir.AluOpType.bypass,
                        replica_groups=[list(range(N_CORES))],
                        ins=[ib.opt()], outs=[ob.opt()])
                    obv = ob[:].rearrange("(r p) x -> p r x", p=128)
                    nc.sync.dma_start(
                        wkt_t[:].rearrange("p (r x) -> p r x", x=NFC * TS),
                        obv[:, :, 0:NFC * TS])
                    nc.scalar.dma_start(
                        wvp_t[:].rearrange("p (r x) -> p r x", x=2 * 520),
                        obv[:, :, NFC * TS:])

                def wkt_cols(jz, gt):
                    """wkt col offset for head-block jz, global token tile gt
                    (128 wide) under either layout."""
                    if kv_split:
                        return (gt // 2) * (NFC * TS) + jz * TS + (gt % 2) * 128
                    return jz * BT + gt * 128

                # ---------- transform units (fine-grained) ----------
                def u_wkt(jz, jp):
                    def f():
                        ps = scps.tile([128, 1024], f32, tag="psc", name="wkps")
                        for half in range(2):
                            jt = jp * 2 + half
                            for jd in range(NDC):
                                nc.tensor.matmul(
                                    ps[:, half * 512: half * 512 + 512],
                                    w_k[:, jd * TF + jz * 128: jd * TF + (jz + 1) * 128],
                                    ht_sb[:, jd * BT + jt * 512: jd * BT + (jt + 1) * 512],
                                    start=(jd == 0), stop=(jd == NDC - 1))
                        nc.scalar.activation(
                            wkt[:, jz * BT + jp * 1024: jz * BT + (jp + 1) * 1024],
                            ps[:, 0:1024], AF.Tanh, bias=b_k[:, jz:jz + 1])
                    return f

                def u_wvp(jt):
                    def f():
                        ps = chainp.tile([128, 512], f32, tag="chain", name="wvps")
                        for jd in range(NDC):
                            nc.tensor.matmul(
                                ps[:, 0:512],
                                ht_sb[:, jd * BT + jt * 128: jd * BT + (jt + 1) * 128],
                                w_v[:, jd * TF:(jd + 1) * TF],
                                start=(jd == 0), stop=False)
                        nc.tensor.matmul(ps[:, 0:512], ones1[0:1, :], bvb[0:1, :],
                                         start=False, stop=True)
                        wvp_z = wvp[:, jt * 520: (jt + 1) * 520].rearrange(
                            "p (z h) -> p z h", h=65)
                        nc.scalar.activation(
                            wvp_z[:, :, 0:64],
                            ps[:, 0:512].rearrange("p (z h) -> p z h", h=64),
                            AF.Tanh)
                    return f

                # ---------- per-chunk phase A units ----------
                def u_qt_dma(st, w):
                    def f():
                        nc.sync.dma_start(
                            st["qt"][:].rearrange("p (c x) -> p c x", x=512)[:, :, 0:w],
                            qt_d[:, st["c0"]:st["c0"] + w].rearrange(
                                "(c p) x -> p c x", p=128))
                    return f

                def u_qg(st, jf, w):
                    def f():
                        ps = chainp.tile([128, 512], f32, tag="chain", name="qgps")
                        for jd in range(NDC):
                            nc.tensor.matmul(
                                ps[:, :w],
                                w_tr[:, jd * TF + jf * 128: jd * TF + (jf + 1) * 128],
                                st["qt"][:, jd * 512: jd * 512 + w],
                                start=(jd == 0), stop=(jd == NDC - 1))
                        nc.scalar.activation(st["qgt"][:, jf * 512: jf * 512 + w],
                                             ps[:, :w], AF.Tanh, bias=b_tr[:, jf:jf + 1])
                    return f

                def u_qtt(st, jz, w):
                    def f():
                        ps = chainp.tile([128, 512], f32, tag="chain", name="qtps")
                        for jf in range(NFC):
                            nc.tensor.matmul(
                                ps[:, :w],
                                w_q[:, jf * TF + jz * 128: jf * TF + (jz + 1) * 128],
                                st["qgt"][:, jf * 512: jf * 512 + w],
                                start=(jf == 0), stop=(jf == NFC - 1))
                        qeng = nc.gpsimd if pool_q else nc.vector
                        qeng.tensor_scalar_add(st["qtt"][:, jz * 512: jz * 512 + w],
                                               ps[:, :w], b_q[:, jz:jz + 1])
                    return f

                def u_qwall(st, cb):
                    def f():
                        ps = chainp.tile([128, 512], f32, tag="chain", name="qwps")
                        for jf in range(NFC):
                            nc.tensor.matmul(
                                ps[:, 0:TF],
                                st["qgt"][:, jf * 512 + cb * 128: jf * 512 + (cb + 1) * 128],
                                w_W[:, jf * TF:(jf + 1) * TF],
                                start=(jf == 0), stop=(jf == NFC - 1))
                        if qwall_act:
                            nc.scalar.activation(
                                st["qwall"][:, cb * TF:(cb + 1) * TF],
                                ps[:, 0:TF], AF.Identity)
                        else:
                            (nc.gpsimd if pool_q else nc.vector).tensor_copy(
                                st["qwall"][:, cb * TF:(cb + 1) * TF],
                                ps[:, 0:TF])
                    return f

                def new_chunk_state(ci):
                    c0, w = C_CHUNKS[ci][0], C_CHUNKS[ci][1]
                    st = {"c0": c0, "w": w, "nb": w // 128, "ci": ci}
                    if ci == chunk_order[0]:
                        st["qt"] = qt0_sb
                    else:
                        st["qt"] = qin.tile([128, NDC * 512], bf, tag="qt", name="qt_sb")
                    st["qgt"] = qg.tile([128, NFC * 512], bf, tag="qgt", name="qgt")
                    st["qtt"] = qg.tile([128, NFC * 512], bf, tag="qtt", name="qtt")
                    st["qwall"] = qg.tile([128, 4 * TF], bf, tag="qwall", name="qwall")
                    st["outts"] = [
                        outp.tile([128, B], f32, tag=f"o{cb}", name=f"outt{cb}")
                        for cb in range(st["nb"])
                    ]
                    return st

                def a_units(st):
                    w = st["w"]
                    us = [u_qt_dma(st, w)]
                    us += [u_qg(st, jf, w) for jf in range(NFC)]
                    us += [u_qtt(st, jz, w) for jz in range(NFC)]
                    us += [u_qwall(st, cb) for cb in range(st["nb"])]
                    return us

                # ---------- attention ----------
                def emit_et(et_ap, psc_ap, zp, wide=True):
                    """exp(16*psc) into et: DVE poly for routed z-pairs,
                    exact ACT Exp otherwise. psc holds s/16. The thin
                    (w=128) chunk keeps exp on ACT when dve_small is False:
                    there DVE is tail-dominated while ACT is underloaded."""
                    if exp16 is not None and zp in dve_zp and (wide or dve_small):
                        nc.vector._custom_dve(
                            exp16, out=et_ap, in0=psc_ap,
                            s0=EXP_C2, s1=EXP_C1, imm2=EXP_C0)
                    else:
                        nc.scalar.activation(et_ap, psc_ap, AF.Exp, scale=16.0)

                # Emission order of heads within a b: DVE-routed heads are
                # spread among ACT heads so both engines chew different psc
                # ring slots CONCURRENTLY (the ring frees slots in emission
                # order, so a run of same-engine tiles serializes on that
                # engine).
                if z_order_cfg is not None:
                    z_order = list(z_order_cfg)
                else:
                    zp_order = [zp for zp in range(NH // 2) if zp in dve_zp] + \
                               [zp for zp in range(NH // 2) if zp not in dve_zp]
                    z_order = [zp * 2 + zi for zp in zp_order for zi in range(2)]
                zp_order = list(dict.fromkeys(z // 2 for z in z_order))

                def emit_scores(st, bb):
                    """scores + exp for all z of (chunk, b); returns ets."""
                    w = st["w"]
                    pr = pop_rate if w > 128 else pop_rate_small
                    ets = {}
                    if w > 128:
                        for z in z_order:
                            jz, hz = z // 2, (z % 2) * 64
                            pair_et = []
                            for half in range(2):
                                psc = scps.tile([128, 1024], f32, tag="psc", name="psc")
                                for slot in range(2):
                                    jt = half * 2 + slot
                                    wc = wkt_cols(jz, bb * 4 + jt)
                                    nc.tensor.matmul(
                                        psc[:, slot * w: slot * w + w],
                                        wkt[hz:hz + 64, wc: wc + 128],
                                        st["qtt"][hz:hz + 64, jz * 512: jz * 512 + w],
                                        start=True, stop=True)
                                et = etp.tile([128, 1024], bf, tag="et", name="et")
                                if a_zero:
                                    emit_et(et[:, 0:2 * w], psc[:, 0:2 * w], z // 2)
                                else:
                                    for slot in range(2):
                                        jt = half * 2 + slot
                                        gt = bb * 4 + jt
                                        nc.scalar.activation(
                                            et[:, slot * w: slot * w + w],
                                            psc[:, slot * w: slot * w + w],
                                            AF.Exp, scale=16.0,
                                            bias=ab_sb[:, gt:gt + 1])
                                pair_et.append(et)
                                pop_fill(pr)
                            ets[z] = pair_et
                    else:
                        for zp in zp_order:
                            psc = scps.tile([128, 1024], f32, tag="psc", name="psc")
                            for zi in range(2):
                                z = zp * 2 + zi
                                jz, hz = z // 2, (z % 2) * 64
                                for jt in range(4):
                                    s = zi * 4 + jt
                                    wc = wkt_cols(jz, bb * 4 + jt)
                                    nc.tensor.matmul(
                                        psc[:, s * 128: (s + 1) * 128],
                                        wkt[hz:hz + 64, wc: wc + 128],
                                        st["qtt"][hz:hz + 64, jz * 512: jz * 512 + w],
                                        start=True, stop=True)
                            et = etp.tile([128, 1024], bf, tag="et", name="et")
                            if a_zero:
                                emit_et(et[:], psc[:], zp, wide=False)
                            else:
                                for zi in range(2):
                                    for jt in range(4):
                                        s = zi * 4 + jt
                                        gt = bb * 4 + jt
                                        nc.scalar.activation(
                                            et[:, s * 128: (s + 1) * 128],
                                            psc[:, s * 128: (s + 1) * 128],
                                            AF.Exp, scale=16.0,
                                            bias=ab_sb[:, gt:gt + 1])
                            ets[zp] = et
                            pop_fill(pr)
                    return ets

                def u_ctx(st, bb, cb, zh, ets, scrs):
                    """context MMs for one (b, c-block, z-half: 4 z); the
                    elementwise tail is deferred via tailq (lagged) so its
                    DVE ops never head-of-line-block later exp work."""
                    w = st["w"]

                    def f():
                        yp = yps.tile([128, 512], f32, tag="chain", name="yp")
                        for zi in range(4):
                            z = zh * 4 + zi
                            for jt in range(4):
                                if w > 128:
                                    half, slot = jt // 2, jt % 2
                                    lhsT = ets[z][half][:, slot * w + cb * 128:
                                                        slot * w + (cb + 1) * 128]
                                else:
                                    s = (z % 2) * 4 + jt
                                    lhsT = ets[z // 2][:, s * 128: (s + 1) * 128]
                                gt = bb * 4 + jt
                                nc.tensor.matmul(
                                    yp[:, zi * 128: zi * 128 + 65],
                                    lhsT,
                                    wvp[:, gt * 520 + z * 65: gt * 520 + (z + 1) * 65],
                                    start=(jt == 0), stop=(jt == 3))

                        def tail():
                            ypz = yp[:].rearrange("p (z c) -> p z c", c=128)
                            prod = prodp.tile([128, 256], f32, tag="prod", name="prod")
                            mul_eng = nc.gpsimd if tail_mul_pool else nc.vector
                            mul_eng.tensor_mul(
                                prod[:].rearrange("p (z h) -> p z h", h=64),
                                ypz[:, :, 0:64],
                                st["qwall"][:, cb * TF + zh * 256:
                                            cb * TF + (zh + 1) * 256].rearrange(
                                    "p (z h) -> p z h", h=64))
                            num = tailp.tile([128, 4], f32, tag="num", name="num")
                            nc.vector.tensor_reduce(
                                num[:], prod[:].rearrange("p (z h) -> p z h", h=64),
                                mybir.AxisListType.X, ALU.add)
                            recd = tailp.tile([128, 4], f32, tag="recd", name="recd")
                            # denominators are sums of positive exps (>= ~1):
                            # the fast approx (~18 bits) is plenty.
                            nc.vector.reciprocal_approx_fast(
                                recd[:], ypz[:, :, 64:65])
                            nc.vector.tensor_mul(scrs[cb][:, zh * 4: (zh + 1) * 4],
                                                 num[:], recd[:])
                            if zh == 1:
                                nc.vector.tensor_reduce(
                                    st["outts"][cb][:, bb:bb + 1],
                                    scrs[cb][:].rearrange("p (o z) -> p o z", o=1),
                                    mybir.AxisListType.X, ALU.add)
                                if bb == B - 1:
                                    nc.sync.dma_start(
                                        out_d[st["c0"] + cb * 128:
                                              st["c0"] + (cb + 1) * 128, :],
                                        st["outts"][cb][:, :])
                        tailq.append(tail)
                    return f

                def ctx_units(st, bb, ets):
                    scrs = [tailp.tile([128, 8], f32, tag=f"scr{cb}", name="scr")
                            for cb in range(st["nb"])]
                    return [u_ctx(st, bb, cb, zh, ets, scrs)
                            for cb in range(st["nb"]) for zh in range(2)]

                # ---------- prologue ----------
                if carry_st0 is not None:
                    # phase-A units (qt DMA, qg, qtt, qwall) were pre-emitted
                    # as fill during the previous rep's last chunk.
                    st0 = carry_st0
                    carry_st0 = None
                    prologue_done = True
                else:
                    st0 = new_chunk_state(chunk_order[0])
                    prologue_done = False
                if (not prologue_done) and rep > 0:
                    u_qt_dma(st0, st0["w"])()
                if kv_split:
                    # local K/V transforms + cross-core gather: inline only on
                    # the cold rep; later reps pre-emit this during rep r-1 so
                    # the collective latency hides behind compute.
                    if kv_cold:
                        for jh in range(2):
                            u_lwkt(jh)()
                        for lt in range(2):
                            u_lwvp(lt)()
                        emit_kv_gather(wkt, wvp)
                    if not prologue_done:
                        for jf in range(NFC):
                            u_qg(st0, jf, st0["w"])()
                        u_qtt(st0, 0, st0["w"])()
                        fill.extend([u_qtt(st0, 1, st0["w"]),
                                     u_qtt(st0, 2, st0["w"]),
                                     u_qtt(st0, 3, st0["w"])])
                else:
                    for jf in range(NFC):
                        u_qg(st0, jf, st0["w"])()
                    u_qtt(st0, 0, st0["w"])()
                    u_wkt(0, 0)()
                    u_wkt(0, 1)()
                    fill.extend([u_wkt(1, 0), u_wkt(1, 1), u_qtt(st0, 1, st0["w"]),
                                 u_wkt(2, 0), u_wkt(2, 1), u_qtt(st0, 2, st0["w"]),
                                 u_wkt(3, 0), u_wkt(3, 1), u_qtt(st0, 3, st0["w"])])
                if not prologue_done:
                    fill.extend([u_qwall(st0, cb) for cb in range(st0["nb"])])
                if not kv_split:
                    fill.extend([u_wvp(jt) for jt in range(4)])

                # ---------- main pipeline ----------
                st = st0
                nst = None
                for pos in range(len(chunk_order)):
                    for bb in range(B):
                        ets = emit_scores(st, bb)
                        fill.extend(ctx_units(st, bb, ets))
                        if pos == 0 and bb < 3 and not kv_split:
                            fill.extend(u_wvp(4 * (bb + 1) + k) for k in range(4))
                        if bb == 0 and pos + 1 < len(chunk_order):
                            nst = new_chunk_state(chunk_order[pos + 1])
                            fill.extend(a_units(nst))
                        cpos, cbb = (carry_pos if carry_pos is not None
                                     else (len(chunk_order) - 1, 0))
                        if (pos == cpos and bb == cbb and rep + 1 < reps):
                            carry_st0 = new_chunk_state(chunk_order[0])
                            fill.extend(a_units(carry_st0))
                        if (kv_split and pos == 1 and bb == 0
                                and rep + 1 < reps):
                            kv_next = (
                                kvp.tile([128, NFC * BT], bf, tag="wkt",
                                         name="wkt"),
                                kvp.tile([128, NTT * 520], bf, tag="wvp",
                                         name="wvp"),
                            )
                            nwkt, nwvp = kv_next
                            fill.extend([u_lwkt(0), u_lwkt(1),
                                         u_lwvp(0), u_lwvp(1)])
                            fill.append(
                                lambda w1=nwkt, w2=nwvp: emit_kv_gather(w1, w2))
                    if boundary_flush or pos + 1 >= len(chunk_order):
                        flush_fill()
                    st = nst
                    nst = None

    nc.compile()
    return nc


# the shipping configuration (used by kernel() and test.py).
# NOTE: GPSIMD cannot access PSUM on real HW (BIR verifier), so the Pool
# offloads (tail_mul_pool / pool_q) must stay False.
BUILD_CFG = dict(dve_zp=(3,), z_order_cfg=(0, 1, 6, 2, 3, 7, 4, 5),
                 tail_mul_pool=False, pool_q=False, tail_lag=0,
                 scps_bufs=3, chainp_bufs=2, yp_bufs=0, kv_split=True,
                 et_bufs=30)


def _get_nc(a_zero: bool, reps: int = 1):
    key = ("nc", a_zero, reps)
    if key not in _CACHE:
        _CACHE[key] = _build(a_zero, reps=reps, **BUILD_CFG)
    return _CACHE[key]


def _prep_inputs(Q, H, a, trans_w, trans_b, q_w, q_b, k_w, k_b, v_w, v_b, W_w):
    """Host-side sharding/layout. Returns (in_maps, a_zero)."""
    a = np.asarray(a, np.float32)
    a_zero = not np.any(a)

    qt_full = np.zeros((D, CP), _BF)
    qt_full[:, :C_FULL] = np.asarray(Q, np.float32).T.astype(_BF)
    ht = np.ascontiguousarray(
        np.asarray(H, np.float32).reshape(BT, D).T.astype(_BF))
    shared = {}
    if not BUILD_CFG.get("kv_split"):
        shared["ht"] = ht
    shared.update({
        "wtr": np.ascontiguousarray(np.asarray(trans_w, np.float32).T.astype(_BF)),
        # q_w/q_b prescaled by 1/16: psc holds s/16 (exp is taken with
        # scale=16 on ACT, or the poly exp16 on DVE which wants s/16).
        "wq": np.ascontiguousarray(
            (np.asarray(q_w, np.float32).T / 16.0).astype(_BF)),
        "wk": np.ascontiguousarray(np.asarray(k_w, np.float32).T.astype(_BF)),
        "wv": np.ascontiguousarray(np.asarray(v_w, np.float32).T.astype(_BF)),
        "ww": np.ascontiguousarray(np.asarray(W_w, np.float32).T.astype(_BF)),
        "btr": np.asarray(trans_b, np.float32),
        "bq": np.asarray(q_b, np.float32) / 16.0,
        "bk": np.asarray(k_b, np.float32),
        "bvb": np.asarray(v_b, np.float32).reshape(1, TF).astype(_BF),
    })
    if not a_zero:
        ab = a.reshape(B, 4, 128).transpose(2, 0, 1).reshape(128, NTT)
        shared["ab"] = np.ascontiguousarray(ab.astype(np.float32))
    in_maps = []
    for c in range(N_CORES):
        m = dict(shared)
        m["qt"] = np.ascontiguousarray(qt_full[:, c * CS:(c + 1) * CS])
        if BUILD_CFG.get("kv_split"):
            ts = BT // N_CORES
            m["hts"] = np.ascontiguousarray(ht[:, c * ts:(c + 1) * ts])
        in_maps.append(m)
    return in_maps, a_zero


def kernel(**inputs) -> np.ndarray:
    from concourse.bass_utils import run_bass_kernel_spmd

    in_maps, a_zero = _prep_inputs(**inputs)
    nc = _get_nc(a_zero)
    res = run_bass_kernel_spmd(nc, in_maps, list(range(N_CORES)))
    out = np.concatenate([res.results[c]["out"] for c in range(N_CORES)], axis=0)
    return np.ascontiguousarray(out.T[:, :C_FULL])

